# revision 1
# baseline (speedup 1.0000x reference)
"""Bass/Trainium2 kernel for nn_Attentioncell (Bahdanau-style attention cell).

Mathematical simplification (verified to rel-err ~6e-7 against the jax
reference): the per-step scores are
    scores[b,l] = (total[b,l,:] + (h @ W2)[b,:]) @ V
               = (total @ V)[b,l] + (h @ W2 @ V)[b]
and softmax over l is invariant to the per-b shift, so the attention
weights are identical for every timestep and independent of h:
    attn = softmax_l(x_static @ (W1 @ V))        (b2, W2, h0 drop out)
    ctx[b,:] = sum_l attn[b,l] * x_static[b,l,:]
    out[b,t,:] = x[b,t,:] @ W3[:D] + ctx[b,:] @ W3[D:] + b3

The scan disappears entirely; the kernel is a handful of matmuls and a
softmax, data-parallel over batch B=32 across 8 NeuronCores (4 per core).

Implementation notes:
  - compute dtype bf16 for everything TensorE/DVE-heavy (measured
    end-to-end rel err ~2.6e-3, well under the 2e-2 gate); exp and all
    reductions accumulate in f32.
  - all big inputs are host-permuted so each SBUF partition's data is
    one contiguous DRAM segment, and each tensor is a single dma_start:
    the first version used 2KB-segment DMAs on one queue and was
    dispatch-bound (~107 GB/s).
  - scores via DVE mul + ACT Copy/accum_out (per-partition sum);
    softmax normalizer Z and context are TensorE partition-reductions
    against a block-diagonal E = exp(scores)*mask.
  - out = sum_j xT_j^T @ W3top_j + Ind5^T @ [c2; b3] accumulated in one
    PSUM bank; Ind5 rows 0..3 are per-batch indicators over the 128
    (b,t) output rows, row 4 is ones (adds b3 to every row).
"""

import numpy as np

B, T, L, S, D = 32, 32, 196, 512, 512
NCORES = 8
BLOC = B // NCORES          # 4 batches per core
BT = BLOC * T               # 128 output rows per core
BL = BLOC * L               # 784 static rows per core
NCH = 7                     # bl chunks
CH = BL // NCH              # 112 rows per chunk

_cache = {}


def _build_graph():
    import concourse.bacc as bacc
    import concourse.tile as tile
    from concourse import mybir

    f32 = mybir.dt.float32
    bf16 = mybir.dt.bfloat16
    nc = bacc.Bacc("TRN2", target_bir_lowering=False, debug=False,
                   num_devices=NCORES)

    # xsp packs [w1vb | xs chunks 0..6 | mask] so every DMA group moves
    # >=2KB-per-partition segments (1KB segments are descriptor-bound).
    XSW = (NCH + 1) * S + NCH * BLOC
    xs_d = nc.dram_tensor("xsp", [CH, XSW], bf16, kind="ExternalInput").ap()
    # w3tx packs [xt slabs | w3t slabs]
    w3t_d = nc.dram_tensor("w3tx", [128, 4 * D + 512], bf16,
                           kind="ExternalInput").ap()
    w3b_d = nc.dram_tensor("w3b", [128, 4 * D], bf16, kind="ExternalInput").ap()
    b3_d = nc.dram_tensor("b3r", [1, D], bf16, kind="ExternalInput").ap()
    ind5_d = nc.dram_tensor("ind5", [5, BT], bf16, kind="ExternalInput").ap()
    id4_d = nc.dram_tensor("id4", [4, 4], bf16, kind="ExternalInput").ap()
    out_d = nc.dram_tensor("out", [BT, D], f32, kind="ExternalOutput").ap()

    with tile.TileContext(nc) as tc:
        with (
            tc.tile_pool(name="big", bufs=1) as big,
            tc.tile_pool(name="small", bufs=1) as small,
            tc.tile_pool(name="scratch", bufs=2) as scratch,
            tc.tile_pool(name="ps_acc", bufs=1, space="PSUM") as ps_acc,
            tc.tile_pool(name="ps_tr", bufs=2, space="PSUM") as ps_tr,
        ):
            xsp = big.tile([CH, XSW], bf16, tag="xsp")
            w1vb = xsp[:, 0:S]
            xs = xsp[:, S:(NCH + 1) * S]
            mask = xsp[:, (NCH + 1) * S:]
            w3tx = big.tile([128, 4 * D + 512], bf16, tag="w3tx")
            xt = w3tx[:, 0:512]
            w3t = w3tx[:, 512:]
            w3b = big.tile([128, 4 * D], bf16, tag="w3b")
            ind5 = small.tile([5, BT], bf16, tag="ind5")
            id4 = small.tile([4, 4], bf16, tag="id4")
            ones = small.tile([CH, 1], f32, tag="ones")
            scores = small.tile([CH, NCH], f32, tag="scores")
            etile = small.tile([CH, NCH], bf16, tag="etile")
            E = small.tile([CH, NCH * BLOC], bf16, tag="E")
            recipZ = small.tile([BLOC, 1], f32, tag="recipZ")
            ctx_sb = small.tile([BLOC, S], bf16, tag="ctx_sb")
            ctxT = small.tile([128, 4 * BLOC], bf16, tag="ctxT")
            rhs5 = small.tile([5, D], bf16, tag="rhs5")
            out_sb = big.tile([BT, D], f32, tag="out_sb")

            # ---- DMA loads. Each issuing engine owns one HW queue
            # (~150 GB/s each), so spread xs quarters across all four
            # queues to land the scores inputs as early as possible;
            # later-needed tensors queue up behind them. ----
            # DMA schedule: every queue ships its xs group first (scores
            # are the critical chain), then xt/W3top pieces (out-matmuls
            # run mid-kernel), then W3bot (needed last, for c2).
            # 2-chunk xs groups: the first mul only needs {w1vb,c0}, so
            # keep that group minimal; every group stays at 2KB segments
            nc.sync.dma_start(xsp[:, 0:2 * S], xs_d[:, 0:2 * S])
            nc.scalar.dma_start(xsp[:, 2 * S:4 * S], xs_d[:, 2 * S:4 * S])
            nc.gpsimd.dma_start(xsp[:, 4 * S:6 * S], xs_d[:, 4 * S:6 * S])
            nc.sync.dma_start(xsp[:, 6 * S:], xs_d[:, 6 * S:])
            nc.scalar.dma_start(w3tx[:, 0:3 * 512], w3t_d[:, 0:3 * 512])
            nc.gpsimd.dma_start(w3tx[:, 3 * 512:], w3t_d[:, 3 * 512:])
            nc.sync.dma_start(w3b[:], w3b_d[:])
            nc.scalar.dma_start(ind5[:], ind5_d[:])
            nc.scalar.dma_start(id4[:], id4_d[:])
            nc.scalar.dma_start(rhs5[4:5, :], b3_d[:])
            nc.vector.memset(ones[:], 1.0)
            out_ps = ps_acc.tile([BT, D], f32, tag="out_ps")

            # ---- scores[bl] = xs[bl,:] . w1v ----
            # (mul on DVE; per-partition sum alternates between ACT
            # Copy+accum_out and DVE tensor_reduce so neither engine
            # serializes the whole phase. tensor_tensor_reduce would
            # fuse this but wedges the DVE on this runtime.)
            # exp / E-build / ctx-matmul are split into two halves so
            # the PE starts accumulating ctx while the second half of
            # scores is still being computed.
            ctx_ps = ps_acc.tile([BLOC, S], f32, tag="ctx_ps")
            z_ps = ps_acc.tile([BLOC, 1], f32, tag="z_ps")

            def scores_chunk(c):
                prod = scratch.tile([CH, S], bf16, tag="prod")
                nc.vector.tensor_mul(prod[:], xs[:, c * S:(c + 1) * S],
                                     w1vb[:])
                if c % 2 == 0:
                    dump = scratch.tile([CH, S], bf16, tag="dump")
                    nc.scalar.activation(dump[:], prod[:],
                                         mybir.ActivationFunctionType.Copy,
                                         accum_out=scores[:, c:c + 1])
                else:
                    nc.vector.tensor_reduce(scores[:, c:c + 1], prod[:],
                                            axis=mybir.AxisListType.X,
                                            op=mybir.AluOpType.add)

            def softmax_half(c0, c1):
                # E[:, 4c+b] = exp(scores[:,c]) * mask for c in [c0,c1)
                n = c1 - c0
                nc.scalar.activation(etile[:, c0:c1], scores[:, c0:c1],
                                     mybir.ActivationFunctionType.Exp)
                nc.vector.tensor_mul(
                    E[:, c0 * BLOC:c1 * BLOC].rearrange(
                        "p (c b) -> p c b", b=BLOC),
                    etile[:, c0:c1].to_broadcast((CH, n, BLOC)),
                    mask[:, c0 * BLOC:c1 * BLOC].rearrange(
                        "p (c b) -> p c b", b=BLOC),
                )
                for c in range(c0, c1):
                    nc.tensor.matmul(ctx_ps[:], E[:, c * BLOC:(c + 1) * BLOC],
                                     xs[:, c * S:(c + 1) * S],
                                     start=(c == 0), stop=(c == NCH - 1))

            scores_chunk(0)
            softmax_half(0, 1)
            scores_chunk(1)
            scores_chunk(2)
            softmax_half(1, 3)
            scores_chunk(3)
            scores_chunk(4)
            softmax_half(3, 5)
            scores_chunk(5)
            scores_chunk(6)
            softmax_half(5, NCH)

            # ---- out partial: x @ W3top (emitted after the ctx chain so
            # the PE prioritizes ctx; accumulation order is irrelevant) ----
            for j in range(4):
                nc.tensor.matmul(out_ps[:], xt[:, j * 128:(j + 1) * 128],
                                 w3t[:, j * D:(j + 1) * D],
                                 start=(j == 0), stop=False,
                                 skip_group_check=True)

            # Z: pre-sum E over chunks on DVE (strided view puts c
            # innermost), then a single [112,4]^T @ ones matmul.
            esum = small.tile([CH, BLOC], f32, tag="esum")
            nc.vector.tensor_reduce(
                esum[:],
                E[:].rearrange("p (c b) -> p b c", b=BLOC),
                axis=mybir.AxisListType.X,
                op=mybir.AluOpType.add)
            nc.tensor.matmul(z_ps[:], esum[:], ones[:], start=True, stop=True)
            nc.vector.reciprocal(recipZ[:], z_ps[:])
            nc.scalar.copy(ctx_sb[:], ctx_ps[:])

            # ---- transpose ctx ([4,512] -> 4x [128,4]) on PE ----
            for j in range(4):
                tr = ps_tr.tile([128, BLOC], bf16, tag="tr")
                nc.tensor.transpose(tr[:], ctx_sb[:, j * 128:(j + 1) * 128],
                                    id4[:])
                nc.vector.tensor_copy(ctxT[:, j * BLOC:(j + 1) * BLOC], tr[:])

            # ---- c2 = ctx @ W3bot (unnormalized), then scale by 1/Z ----
            c2_ps = ps_acc.tile([BLOC, D], f32, tag="c2_ps")
            for j in range(4):
                nc.tensor.matmul(c2_ps[:], ctxT[:, j * BLOC:(j + 1) * BLOC],
                                 w3b[:, j * D:(j + 1) * D],
                                 start=(j == 0), stop=(j == 3))
            nc.vector.tensor_scalar_mul(rhs5[0:4, :], c2_ps[:], recipZ[:])

            # ---- out += Ind5^T @ [c2; b3], in two row halves so the
            # copy-out + DMA of half 0 overlaps the matmul of half 1
            # (row split keeps the out DMA at 2KB/partition segments) ----
            H = BT // 2
            for h in range(2):
                sl = slice(h * H, (h + 1) * H)
                nc.tensor.matmul(out_ps[sl, :], ind5[:, sl], rhs5[:],
                                 start=False, stop=(h == 1),
                                 skip_group_check=True)
                nc.scalar.copy(out_sb[sl, :], out_ps[sl, :])
                eng = nc.sync if h == 0 else nc.scalar
                eng.dma_start(out_d[sl, :], out_sb[sl, :])

    nc.compile()
    return nc


def _get_graph():
    if "nc" not in _cache:
        _cache["nc"] = _build_graph()
    return _cache["nc"]


def _consts():
    if "consts" in _cache:
        return _cache["consts"]
    import ml_dtypes
    bf = ml_dtypes.bfloat16
    ind5 = np.zeros((5, BT), np.float32)
    for b in range(BLOC):
        ind5[b, b * T:(b + 1) * T] = 1.0
    ind5[4, :] = 1.0
    mask = np.zeros((CH, NCH, BLOC), np.float32)
    for c in range(NCH):
        for p in range(CH):
            mask[p, c, (c * CH + p) // L] = 1.0
    consts = {
        "ind5": np.ascontiguousarray(ind5.astype(bf)),
        "id4": np.ascontiguousarray(np.eye(4).astype(bf)),
        "_mask": mask.reshape(CH, NCH * BLOC).astype(np.float32),
    }
    _cache["consts"] = consts
    return consts


def kernel(x, x_static, h0, W1, W2, W3, b2, b3, V, **_unused):
    import ml_dtypes
    from concourse.bass_utils import run_bass_kernel_spmd
    bf = ml_dtypes.bfloat16

    x = np.asarray(x, np.float32)
    x_static = np.asarray(x_static, np.float32)
    W1 = np.asarray(W1, np.float32)
    W3 = np.asarray(W3, np.float32)
    b3 = np.asarray(b3, np.float32)
    V = np.asarray(V, np.float32)

    # Host-side weight folding (weights are per-model constants).
    w1v = (W1 @ V).reshape(-1).astype(np.float32)           # [S]
    w1vb = np.broadcast_to(w1v, (CH, S))
    # per-partition-contiguous permuted layouts (one big DMA segment
    # per partition):
    w3t = (W3[:D].reshape(4, 128, D).transpose(1, 0, 2)
           .reshape(128, 4 * D))
    w3b = np.ascontiguousarray(
        W3[D:].reshape(4, 128, D).transpose(1, 0, 2).reshape(128, 4 * D)
        .astype(bf))
    b3r = np.ascontiguousarray(b3.reshape(1, D).astype(bf))
    consts = _consts()

    nc = _get_graph()
    in_maps = []
    for i in range(NCORES):
        sl = slice(i * BLOC, (i + 1) * BLOC)
        xs_l = x_static[sl].reshape(BL, S)
        xs_p = xs_l.reshape(NCH, CH, S).transpose(1, 0, 2).reshape(CH, NCH * S)
        xsp = np.ascontiguousarray(
            np.concatenate([w1vb, xs_p, consts["_mask"]], axis=1).astype(bf))
        xt_l = x[sl].reshape(BT, D).T                        # [512, 128]
        xt_p = (xt_l.reshape(4, 128, 128).transpose(1, 0, 2)
                .reshape(128, 512))
        w3tx = np.ascontiguousarray(
            np.concatenate([xt_p, w3t], axis=1).astype(bf))
        in_maps.append({
            "xsp": xsp, "w3tx": w3tx,
            "w3b": w3b, "b3r": b3r,
            "ind5": consts["ind5"], "id4": consts["id4"],
        })
    res = run_bass_kernel_spmd(nc, in_maps, core_ids=list(range(NCORES)))
    out = np.empty((B, T, D), np.float32)
    for i in range(NCORES):
        out[i * BLOC:(i + 1) * BLOC] = res.results[i]["out"].reshape(BLOC, T, D)
    return out



# revision 4
# speedup vs baseline: 1.0504x; 1.0504x over previous
"""Bass/Trainium2 kernel for nn_Attentioncell (Bahdanau-style attention cell).

Mathematical simplification (rel-err ~6e-7 vs the jax reference): the
per-step scores are
    scores[b,l] = (total[b,l,:] + (h @ W2)[b,:]) @ V
               = (total @ V)[b,l] + (h @ W2 @ V)[b]
and softmax over l is invariant to the per-b shift, so the attention
weights are identical for every timestep and independent of h:
    attn = softmax_l(x_static @ (W1 @ V))        (b2, W2, h0 drop out)
    ctx[b,:] = sum_l attn[b,l] * x_static[b,l,:]
    out[b,t,:] = x[b,t,:] @ W3[:D] + ctx[b,:] @ W3[D:] + b3

Additional host-side weight folding: with w1v = W1@V,
    scores[l] = sum_s (x_static[l,s] * w1v[s])   = rowsum(xsw)
    ctx' = E^T @ xsw        (xsw = x_static * w1v, elementwise over s)
    ctx' @ (W3[D:] / w1v)  == ctx @ W3[D:]       (exact algebra)
so the device never multiplies by w1v: scores are plain row-sums of the
pre-scaled xsw, and W3bot is divided by w1v on the host.  Each product
term in the c2 GEMM has exactly the same magnitude as before, so the
bf16 error profile is unchanged (min |w1v| ~ 7e-5 -> max |W3bot'| ~2e3,
comfortably inside bf16 range).

Data-parallel over batch B=32 across 8 NeuronCores (4 per core).

Schedule notes (from perfetto trace analysis of the previous version):
  - PE has a DVFS p-state ramp (0.65 -> 1.2 -> 2.4 GHz after ~3us of
    continuous busy).  Dummy matmuls on a memset tile keep the PE busy
    through the ~2.6us DMA-latency head so real matmuls run at high
    clock; extra dummies are placed in known schedule bubbles.
  - 4 DMA queues (sync/scalar/gpsimd/vector), chunk order chosen by
    consumption deadline; per-DMA latency is ~2.2us fixed (descriptor
    gen + DGE start + completion-semaphore propagation) + transfer.
  - reduces: DVE tensor_reduce for chunks {0,2,4,5,6} (~.85us each),
    ACT Copy+accum for {1,3} (~1.2us each); exps in 4 waves on ACT;
    E-builds (exp * block-diagonal mask) on the otherwise-idle GpSimd.
  - Z = sum_l E via 7 tiny PE matmuls against a ones column riding the
    ctx accumulation; 1/Z is folded into the Ind5 indicator matrix
    (IndC = ind5 * recipZ) so the ctx PSUM->SBUF copy does not wait on
    the reciprocal.
  - out is produced in bf16 (halves the output DMA) and upconverted on
    the host; rel-err stays ~1e-3 vs the 2e-2 gate.
"""

import numpy as np

B, T, L, S, D = 32, 32, 196, 512, 512
NCORES = 8
BLOC = B // NCORES          # 4 batches per core
BT = BLOC * T               # 128 output rows per core
BL = BLOC * L               # 784 static rows per core
NCH = 7                     # xsw chunks
CH = BL // NCH              # 112 rows per chunk
MW = NCH * BLOC             # 28 mask columns

# xsp column layout: [c0 | c1 | mask | c2 | c3 | c4 | c5 | c6]
# (mask rides in the same DMA group as c1 so it lands early for the
# first E-build).
_CHOFF = [0, S, 2 * S + MW, 3 * S + MW, 4 * S + MW, 5 * S + MW, 6 * S + MW]
MASKOFF = 2 * S
XSW = 7 * S + MW            # 3612

# consts layout: [ind5 (128) | id4 (4) | rhs5 region (512)] on 5 partitions.
# rhs5 region row 4 = b3 (via DMA); rows 0..3 overwritten on-device by c2.
CW = 128 + 4 + 512

_cache = {}


def _build_graph():
    import concourse.bacc as bacc
    import concourse.tile as tile
    from concourse import mybir

    f32 = mybir.dt.float32
    bf16 = mybir.dt.bfloat16
    nc = bacc.Bacc("TRN2", target_bir_lowering=False, debug=False,
                   num_devices=NCORES)

    xs_d = nc.dram_tensor("xsp", [CH, XSW], bf16, kind="ExternalInput").ap()
    w3t_d = nc.dram_tensor("w3tx", [128, 512 + 4 * D], bf16,
                           kind="ExternalInput").ap()
    w3b_d = nc.dram_tensor("w3b", [128, 4 * D], bf16, kind="ExternalInput").ap()
    cst_d = nc.dram_tensor("cst", [5, CW], bf16, kind="ExternalInput").ap()
    out_d = nc.dram_tensor("out", [BT, D], bf16, kind="ExternalOutput").ap()

    with tile.TileContext(nc) as tc:
        with (
            tc.tile_pool(name="big", bufs=1) as big,
            tc.tile_pool(name="small", bufs=1) as small,
            tc.tile_pool(name="scratch", bufs=2) as scratch,
            tc.tile_pool(name="ps_acc", bufs=1, space="PSUM") as ps_acc,
            tc.tile_pool(name="ps_tr", bufs=2, space="PSUM") as ps_tr,
        ):
            xsp = big.tile([CH, XSW], bf16, tag="xsp")
            mask = xsp[:, MASKOFF:MASKOFF + MW]
            w3tx = big.tile([128, 512 + 4 * D], bf16, tag="w3tx")
            xt = w3tx[:, 0:512]
            w3t = w3tx[:, 512:]
            w3b = big.tile([128, 4 * D], bf16, tag="w3b")
            cst = small.tile([5, CW], bf16, tag="cst")
            ind5 = cst[:, 0:128]
            id4 = cst[0:4, 128:132]
            rhs5 = cst[:, 132:132 + D]
            dummy = big.tile([128, 512], bf16, tag="dummy")
            ones = small.tile([CH, 1], bf16, tag="ones")
            scores = small.tile([CH, NCH], f32, tag="scores")
            etile = small.tile([CH, NCH], bf16, tag="etile")
            E = small.tile([CH, MW], bf16, tag="E")
            recipZ5 = small.tile([5, 1], f32, tag="recipZ5")
            IndC = small.tile([5, BT], bf16, tag="IndC")
            ctx_sb = small.tile([BLOC, S], bf16, tag="ctx_sb")
            ctxT = small.tile([128, 4 * BLOC], bf16, tag="ctxT")
            out_sb = big.tile([BT, D], bf16, tag="out_sb")

            # ---- memsets first (dummy feeds the PE warmup stream) ----
            nc.gpsimd.memset(dummy[:], 0.0)
            nc.gpsimd.memset(ones[:], 1.0)
            nc.vector.memset(recipZ5[:], 1.0)   # rows 0..3 overwritten by recip

            # ---- DMA loads: 4 engine queues, ordered by deadline ----
            def chunk(c):
                return xsp[:, _CHOFF[c]:_CHOFF[c] + S]

            # (only SP/ACT/GpSimd own DMA queues; DVE stays compute-only)
            nc.sync.dma_start(xsp[:, 0:S], xs_d[:, 0:S])                 # c0
            nc.scalar.dma_start(xsp[:, S:S + S + MW],
                                xs_d[:, S:S + S + MW])                   # c1+mask
            nc.gpsimd.dma_start(xsp[:, _CHOFF[2]:_CHOFF[2] + S],
                                xs_d[:, _CHOFF[2]:_CHOFF[2] + S])        # c2
            nc.sync.dma_start(xsp[:, _CHOFF[3]:_CHOFF[3] + S],
                              xs_d[:, _CHOFF[3]:_CHOFF[3] + S])          # c3
            nc.scalar.dma_start(xsp[:, _CHOFF[4]:_CHOFF[4] + S],
                                xs_d[:, _CHOFF[4]:_CHOFF[4] + S])        # c4
            nc.gpsimd.dma_start(xsp[:, _CHOFF[5]:_CHOFF[5] + S],
                                xs_d[:, _CHOFF[5]:_CHOFF[5] + S])        # c5
            nc.sync.dma_start(w3tx[:, 0:512], w3t_d[:, 0:512])           # xt
            nc.gpsimd.dma_start(xsp[:, _CHOFF[6]:_CHOFF[6] + S],
                                xs_d[:, _CHOFF[6]:_CHOFF[6] + S])        # c6
            nc.sync.dma_start(w3tx[:, 512:512 + 2 * D],
                              w3t_d[:, 512:512 + 2 * D])                 # w3t01
            nc.scalar.dma_start(w3b[:, 0:2 * D], w3b_d[:, 0:2 * D])      # w3b01
            nc.gpsimd.dma_start(w3b[:, 2 * D:], w3b_d[:, 2 * D:])        # w3b23
            nc.sync.dma_start(cst[:], cst_d[:])                          # consts
            nc.gpsimd.dma_start(w3tx[:, 512 + 2 * D:],
                                w3t_d[:, 512 + 2 * D:])                  # w3t23

            out_ps = ps_acc.tile([BT, D], f32, tag="out_ps")
            ctx_ps = ps_acc.tile([BLOC, S], f32, tag="ctx_ps")
            z_ps = ps_acc.tile([BLOC, 1], f32, tag="z_ps")
            c2_ps = ps_acc.tile([BLOC, D], f32, tag="c2_ps")
            dm_ps = ps_acc.tile([128, 512], f32, tag="dm_ps")

            # ---- PE warmup: keep the array busy through the DMA head so
            # the p-state ramps to full clock before real work arrives ----
            def dummy_mm(n):
                for _ in range(n):
                    nc.tensor.matmul(dm_ps[:], dummy[:, 0:128], dummy[:],
                                     start=True, stop=True,
                                     skip_group_check=True)

            dummy_mm(10)

            # ---- per-chunk score reduces (DVE {0,2,4,5,6}, ACT {1,3}) ----
            def dve_reduce(c):
                nc.vector.tensor_reduce(scores[:, c:c + 1], chunk(c),
                                        axis=mybir.AxisListType.X,
                                        op=mybir.AluOpType.add)

            def act_reduce(c):
                dump = scratch.tile([CH, S], bf16, tag="dump")
                nc.scalar.activation(dump[:], chunk(c),
                                     mybir.ActivationFunctionType.Copy,
                                     accum_out=scores[:, c:c + 1])

            for c in (0, 2, 4, 5, 6):
                dve_reduce(c)
            for c in (1, 3):
                act_reduce(c)

            # ---- exp waves (ACT) + E-builds (GpSimd) + ctx/z matmuls ----
            def exp_wave(a, b):
                nc.scalar.activation(etile[:, a:b], scores[:, a:b],
                                     mybir.ActivationFunctionType.Exp)

            def ebuild(a, b):
                n = b - a
                nc.gpsimd.tensor_mul(
                    E[:, a * BLOC:b * BLOC].rearrange(
                        "p (c b) -> p c b", b=BLOC),
                    etile[:, a:b].to_broadcast((CH, n, BLOC)),
                    mask[:, a * BLOC:b * BLOC].rearrange(
                        "p (c b) -> p c b", b=BLOC),
                )

            def ctx_mm(c):
                nc.tensor.matmul(ctx_ps[:], E[:, c * BLOC:(c + 1) * BLOC],
                                 chunk(c), start=(c == 0), stop=(c == NCH - 1))
                nc.tensor.matmul(z_ps[:], E[:, c * BLOC:(c + 1) * BLOC],
                                 ones[:], start=(c == 0), stop=(c == NCH - 1))

            def out_top(j):
                nc.tensor.matmul(out_ps[:], xt[:, j * 128:(j + 1) * 128],
                                 w3t[:, j * D:(j + 1) * D],
                                 start=(j == 0), stop=False,
                                 skip_group_check=True)

            exp_wave(0, 1)
            ebuild(0, 1)
            ctx_mm(0)
            exp_wave(1, 2)
            ebuild(1, 2)
            ctx_mm(1)
            out_top(0)
            out_top(1)
            out_top(2)
            exp_wave(2, 4)
            ebuild(2, 4)
            ctx_mm(2)
            ctx_mm(3)
            out_top(3)
            dummy_mm(2)
            exp_wave(4, 7)
            ebuild(4, 7)
            ctx_mm(4)
            ctx_mm(5)
            ctx_mm(6)

            # ---- 1/Z -> IndC (off the ctx critical path) ----
            nc.vector.reciprocal(recipZ5[0:4, :], z_ps[:])
            nc.vector.tensor_scalar_mul(IndC[:], ind5[:], recipZ5[:])

            # ---- ctx PSUM->SBUF, transpose to [128,4] slabs, c2 GEMM ----
            nc.scalar.copy(ctx_sb[:], ctx_ps[:])
            dummy_mm(3)
            for j in range(4):
                tr = ps_tr.tile([128, BLOC], bf16, tag="tr")
                nc.tensor.transpose(tr[:], ctx_sb[:, j * 128:(j + 1) * 128],
                                    id4)
                nc.vector.tensor_copy(ctxT[:, j * BLOC:(j + 1) * BLOC], tr[:])
            for j in range(4):
                nc.tensor.matmul(c2_ps[:], ctxT[:, j * BLOC:(j + 1) * BLOC],
                                 w3b[:, j * D:(j + 1) * D],
                                 start=(j == 0), stop=(j == 3))

            # rhs5 rows 0..3 = unnormalized c2 (split copy ACT/DVE);
            # row 4 = b3 (already there via the consts DMA).
            nc.scalar.copy(rhs5[0:4, 0:256], c2_ps[:, 0:256])
            nc.vector.tensor_copy(rhs5[0:4, 256:512], c2_ps[:, 256:512])
            dummy_mm(2)

            # ---- out += IndC^T @ [c2; b3] (normalization riding IndC),
            # two row halves so copy-out/DMA of half 0 overlaps half 1 ----
            H = BT // 2
            for h in range(2):
                sl = slice(h * H, (h + 1) * H)
                nc.tensor.matmul(out_ps[sl, :], IndC[:, sl], rhs5[:],
                                 start=False, stop=(h == 1),
                                 skip_group_check=True)
                if h == 0:
                    nc.scalar.copy(out_sb[sl, :], out_ps[sl, :])
                    nc.sync.dma_start(out_d[sl, :], out_sb[sl, :])
                else:
                    nc.vector.tensor_copy(out_sb[sl, :], out_ps[sl, :])
                    nc.gpsimd.dma_start(out_d[sl, :], out_sb[sl, :])

    nc.compile()
    return nc


def _get_graph():
    if "nc" not in _cache:
        _cache["nc"] = _build_graph()
    return _cache["nc"]


def _consts():
    if "consts" in _cache:
        return _cache["consts"]
    import ml_dtypes
    bf = ml_dtypes.bfloat16
    mask = np.zeros((CH, NCH, BLOC), np.float32)
    for c in range(NCH):
        for p in range(CH):
            mask[p, c, (c * CH + p) // L] = 1.0
    _cache["consts"] = {"_mask": mask.reshape(CH, MW)}
    return _cache["consts"]


def kernel(x, x_static, h0, W1, W2, W3, b2, b3, V, **_unused):
    import ml_dtypes
    from concourse.bass_utils import run_bass_kernel_spmd
    bf = ml_dtypes.bfloat16

    x = np.asarray(x, np.float32)
    x_static = np.asarray(x_static, np.float32)
    W1 = np.asarray(W1, np.float32)
    W3 = np.asarray(W3, np.float32)
    b3 = np.asarray(b3, np.float32)
    V = np.asarray(V, np.float32)

    # Host-side weight folding (weights are per-model constants).
    w1v = (W1 @ V).reshape(-1)                               # [S]
    w3t = (W3[:D].reshape(4, 128, D).transpose(1, 0, 2)
           .reshape(128, 4 * D))
    w3b = np.ascontiguousarray(
        (W3[D:] / w1v[:, None]).reshape(4, 128, D).transpose(1, 0, 2)
        .reshape(128, 4 * D).astype(bf))
    consts = _consts()
    cst = np.zeros((5, CW), np.float32)
    for b in range(BLOC):
        cst[b, b * T:(b + 1) * T] = 1.0                      # ind5 rows
    cst[4, 0:BT] = 1.0
    cst[0:4, 128:132] = np.eye(4)                            # id4
    cst[4, 132:132 + D] = b3                                 # b3 row
    cst = np.ascontiguousarray(cst.astype(bf))

    nc = _get_graph()
    in_maps = []
    for i in range(NCORES):
        sl = slice(i * BLOC, (i + 1) * BLOC)
        xsw = (x_static[sl].reshape(BL, S) * w1v[None, :])
        xs_p = xsw.reshape(NCH, CH, S).transpose(1, 0, 2)    # [CH, NCH, S]
        xsp = np.empty((CH, XSW), np.float32)
        for c in range(NCH):
            xsp[:, _CHOFF[c]:_CHOFF[c] + S] = xs_p[:, c]
        xsp[:, MASKOFF:MASKOFF + MW] = consts["_mask"]
        xsp = np.ascontiguousarray(xsp.astype(bf))
        xt_l = x[sl].reshape(BT, D).T                        # [512, 128]
        xt_p = (xt_l.reshape(4, 128, 128).transpose(1, 0, 2)
                .reshape(128, 512))
        w3tx = np.ascontiguousarray(
            np.concatenate([xt_p, w3t], axis=1).astype(bf))
        in_maps.append({
            "xsp": xsp, "w3tx": w3tx, "w3b": w3b, "cst": cst,
        })
    res = run_bass_kernel_spmd(nc, in_maps, core_ids=list(range(NCORES)))
    out = np.empty((B, T, D), np.float32)
    for i in range(NCORES):
        out[i * BLOC:(i + 1) * BLOC] = (
            res.results[i]["out"].astype(np.float32).reshape(BLOC, T, D))
    return out


# revision 7
# speedup vs baseline: 1.0565x; 1.0058x over previous
"""Bass/Trainium2 kernel for nn_Attentioncell (Bahdanau-style attention cell).

Mathematical simplification (rel-err ~6e-7 vs the jax reference): the
per-step scores are
    scores[b,l] = (total[b,l,:] + (h @ W2)[b,:]) @ V
               = (total @ V)[b,l] + (h @ W2 @ V)[b]
and softmax over l is invariant to the per-b shift, so the attention
weights are identical for every timestep and independent of h:
    attn = softmax_l(x_static @ (W1 @ V))        (b2, W2, h0 drop out)
    ctx[b,:] = sum_l attn[b,l] * x_static[b,l,:]
    out[b,t,:] = x[b,t,:] @ W3[:D] + ctx[b,:] @ W3[D:] + b3

Additional host-side weight folding: with w1v = W1@V,
    scores[l] = sum_s (x_static[l,s] * w1v[s])   = rowsum(xsw)
    ctx' = E^T @ xsw        (xsw = x_static * w1v, elementwise over s)
    ctx' @ (W3[D:] / w1v)  == ctx @ W3[D:]       (exact algebra)
so the device never multiplies by w1v: scores are plain row-sums of the
pre-scaled xsw, and W3bot is divided by w1v on the host.  Each product
term in the c2 GEMM has exactly the same magnitude as before, so the
bf16 error profile is unchanged (min |w1v| ~ 7e-5 -> max |W3bot'| ~2e3,
comfortably inside bf16 range).

Data-parallel over batch B=32 across 8 NeuronCores (4 per core).

Schedule notes (from perfetto trace analysis of the previous version):
  - PE has a DVFS p-state ramp (0.65 -> 1.2 -> 2.4 GHz after ~3us of
    continuous busy).  Dummy matmuls on a memset tile keep the PE busy
    through the ~2.6us DMA-latency head so real matmuls run at high
    clock; extra dummies are placed in known schedule bubbles.
  - 4 DMA queues (sync/scalar/gpsimd/vector), chunk order chosen by
    consumption deadline; per-DMA latency is ~2.2us fixed (descriptor
    gen + DGE start + completion-semaphore propagation) + transfer.
  - reduces: DVE tensor_reduce for chunks {0,2,4,5,6} (~.85us each),
    ACT Copy+accum for {1,3} (~1.2us each); exps in 4 waves on ACT;
    E-builds (exp * block-diagonal mask) on the otherwise-idle GpSimd.
  - Z = sum_l E via 7 tiny PE matmuls against a ones column riding the
    ctx accumulation; 1/Z is folded into the Ind5 indicator matrix
    (IndC = ind5 * recipZ) so the ctx PSUM->SBUF copy does not wait on
    the reciprocal.
  - out is produced in bf16 (halves the output DMA) and upconverted on
    the host; rel-err stays ~1e-3 vs the 2e-2 gate.
"""

import numpy as np

B, T, L, S, D = 32, 32, 196, 512, 512
NCORES = 8
BLOC = B // NCORES          # 4 batches per core
BT = BLOC * T               # 128 output rows per core
BL = BLOC * L               # 784 static rows per core
NCH = 7                     # xsw chunks
CH = BL // NCH              # 112 rows per chunk
MW = NCH * BLOC             # 28 mask columns

# xsp column layout: [c0 | c1 | mask | c2 | c3 | c4 | c5 | c6]
# (mask rides in the same DMA group as c1 so it lands early for the
# first E-build).
_CHOFF = [0, S, 2 * S + MW, 3 * S + MW, 4 * S + MW, 5 * S + MW, 6 * S + MW]
MASKOFF = 2 * S
XSW = 7 * S + MW            # 3612

# consts layout: [ind5 (128) | id4 (4) | rhs5 region (512)] on 5 partitions.
# rhs5 region row 4 = b3 (via DMA); rows 0..3 overwritten on-device by c2.
CW = 128 + 4 + 512

_cache = {}


def _build_graph():
    import concourse.bacc as bacc
    import concourse.tile as tile
    from concourse import mybir

    f32 = mybir.dt.float32
    bf16 = mybir.dt.bfloat16
    nc = bacc.Bacc("TRN2", target_bir_lowering=False, debug=False,
                   num_devices=NCORES)

    xs_d = nc.dram_tensor("xsp", [CH, XSW], bf16, kind="ExternalInput").ap()
    w3t_d = nc.dram_tensor("w3tx", [128, 512 + 4 * D], bf16,
                           kind="ExternalInput").ap()
    w3b_d = nc.dram_tensor("w3b", [128, 4 * D], bf16, kind="ExternalInput").ap()
    cst_d = nc.dram_tensor("cst", [5, CW], bf16, kind="ExternalInput").ap()
    out_d = nc.dram_tensor("out", [BT, D], bf16, kind="ExternalOutput").ap()

    with tile.TileContext(nc) as tc:
        with (
            tc.tile_pool(name="big", bufs=1) as big,
            tc.tile_pool(name="small", bufs=1) as small,
            tc.tile_pool(name="scratch", bufs=2) as scratch,
            tc.tile_pool(name="ps_acc", bufs=1, space="PSUM") as ps_acc,
            tc.tile_pool(name="ps_tr", bufs=2, space="PSUM") as ps_tr,
        ):
            xsp = big.tile([CH, XSW], bf16, tag="xsp")
            mask = xsp[:, MASKOFF:MASKOFF + MW]
            w3tx = big.tile([128, 512 + 4 * D], bf16, tag="w3tx")
            xt = w3tx[:, 0:512]
            w3t = w3tx[:, 512:]
            w3b = big.tile([128, 4 * D], bf16, tag="w3b")
            cst = small.tile([5, CW], bf16, tag="cst")
            ind5 = cst[:, 0:128]
            id4 = cst[0:4, 128:132]
            rhs5 = cst[:, 132:132 + D]
            dummy = big.tile([128, 512], bf16, tag="dummy")
            ones = small.tile([CH, 1], bf16, tag="ones")
            scores = small.tile([CH, NCH], f32, tag="scores")
            etile = small.tile([CH, NCH], bf16, tag="etile")
            E = small.tile([CH, MW], bf16, tag="E")
            recipZ5 = small.tile([5, 1], f32, tag="recipZ5")
            IndC = small.tile([5, BT], bf16, tag="IndC")
            ctx_sb = small.tile([BLOC, S], bf16, tag="ctx_sb")
            ctxT = small.tile([128, 4 * BLOC], bf16, tag="ctxT")
            out_sb = big.tile([BT, D], bf16, tag="out_sb")

            # ---- memsets first (dummy feeds the PE warmup stream; on DVE
            # so the GpSimd DMA queue starts immediately) ----
            nc.vector.memset(dummy[:], 0.0)
            nc.vector.memset(recipZ5[:], 1.0)   # rows 0..3 overwritten by recip

            # ---- DMA loads. The 3 queues (SP/ACT/GpSimd) share ~300GB/s
            # of HBM, so the ~2MB input stream takes ~7us no matter how it
            # is split; ORDER is what matters: score chunks first (compute
            # starts ~9.8us), then w3b (c2 GEMM ~15us), then xt/w3t (final
            # out matmuls ~17us).  Chunk pairs keep 2KB DMA segments. ----
            def chunk(c):
                return xsp[:, _CHOFF[c]:_CHOFF[c] + S]

            nc.sync.dma_start(xsp[:, 0:S], xs_d[:, 0:S])                 # c0
            nc.scalar.dma_start(xsp[:, S:S + S + MW],
                                xs_d[:, S:S + S + MW])                   # c1+mask
            nc.gpsimd.dma_start(xsp[:, _CHOFF[2]:_CHOFF[2] + S],
                                xs_d[:, _CHOFF[2]:_CHOFF[2] + S])        # c2
            nc.sync.dma_start(cst[:], cst_d[:])                          # consts
            nc.sync.dma_start(xsp[:, _CHOFF[3]:_CHOFF[3] + 2 * S],
                              xs_d[:, _CHOFF[3]:_CHOFF[3] + 2 * S])      # c3+c4
            nc.scalar.dma_start(xsp[:, _CHOFF[5]:_CHOFF[5] + 2 * S],
                                xs_d[:, _CHOFF[5]:_CHOFF[5] + 2 * S])    # c5+c6
            nc.gpsimd.dma_start(w3tx[:, 0:1024], w3t_d[:, 0:1024])       # xt+w3t0
            nc.sync.dma_start(w3b[:, 0:2 * D], w3b_d[:, 0:2 * D])        # w3b01
            nc.scalar.dma_start(w3b[:, 2 * D:], w3b_d[:, 2 * D:])        # w3b23
            nc.gpsimd.dma_start(w3tx[:, 1024:1024 + 2 * D],
                                w3t_d[:, 1024:1024 + 2 * D])             # w3t12
            nc.sync.dma_start(w3tx[:, 1024 + 2 * D:],
                              w3t_d[:, 1024 + 2 * D:])                   # w3t3
            nc.gpsimd.memset(ones[:], 1.0)

            out_ps = ps_acc.tile([BT, D], f32, tag="out_ps")
            ctx_ps = ps_acc.tile([BLOC, S], f32, tag="ctx_ps")
            z_ps = ps_acc.tile([BLOC, 1], f32, tag="z_ps")
            c2_ps = ps_acc.tile([BLOC, D], f32, tag="c2_ps")
            dm_ps = ps_acc.tile([128, 512], f32, tag="dm_ps")

            # ---- PE warmup: keep the array busy through the DMA head so
            # the p-state ramps to full clock before real work arrives ----
            def dummy_mm(n):
                for _ in range(n):
                    nc.tensor.matmul(dm_ps[:], dummy[:, 0:128], dummy[:],
                                     start=True, stop=True,
                                     skip_group_check=True)

            dummy_mm(10)

            # ---- per-chunk score reduces (DVE {0,2,4,5,6}, ACT {1,3}) ----
            def dve_reduce(c):
                nc.vector.tensor_reduce(scores[:, c:c + 1], chunk(c),
                                        axis=mybir.AxisListType.X,
                                        op=mybir.AluOpType.add)

            def act_reduce(c):
                dump = scratch.tile([CH, S], bf16, tag="dump")
                nc.scalar.activation(dump[:], chunk(c),
                                     mybir.ActivationFunctionType.Copy,
                                     accum_out=scores[:, c:c + 1])

            for c in (0, 2, 4, 5, 6):
                dve_reduce(c)
            for c in (1, 3):
                act_reduce(c)

            # ---- exp waves (ACT) + E-builds (GpSimd) + ctx/z matmuls ----
            def exp_wave(a, b):
                nc.scalar.activation(etile[:, a:b], scores[:, a:b],
                                     mybir.ActivationFunctionType.Exp)

            def ebuild(a, b):
                n = b - a
                nc.gpsimd.tensor_mul(
                    E[:, a * BLOC:b * BLOC].rearrange(
                        "p (c b) -> p c b", b=BLOC),
                    etile[:, a:b].to_broadcast((CH, n, BLOC)),
                    mask[:, a * BLOC:b * BLOC].rearrange(
                        "p (c b) -> p c b", b=BLOC),
                )

            def ctx_mm(c):
                nc.tensor.matmul(ctx_ps[:], E[:, c * BLOC:(c + 1) * BLOC],
                                 chunk(c), start=(c == 0), stop=(c == NCH - 1))
                nc.tensor.matmul(z_ps[:], E[:, c * BLOC:(c + 1) * BLOC],
                                 ones[:], start=(c == 0), stop=(c == NCH - 1))

            def out_top(j):
                nc.tensor.matmul(out_ps[:], xt[:, j * 128:(j + 1) * 128],
                                 w3t[:, j * D:(j + 1) * D],
                                 start=(j == 0), stop=False,
                                 skip_group_check=True)

            # wave structure: last wave kept small so the serial tail
            # (ctx copy -> transpose -> c2 -> rhs5 -> final mm) starts ASAP
            exp_wave(0, 1)
            ebuild(0, 1)
            ctx_mm(0)
            exp_wave(1, 2)
            ebuild(1, 2)
            ctx_mm(1)
            out_top(0)
            dummy_mm(2)
            exp_wave(2, 4)
            ebuild(2, 4)
            ctx_mm(2)
            ctx_mm(3)
            out_top(1)
            exp_wave(4, 6)
            ebuild(4, 6)
            ctx_mm(4)
            ctx_mm(5)
            out_top(2)
            exp_wave(6, 7)
            ebuild(6, 7)
            ctx_mm(6)
            out_top(3)

            # ---- 1/Z -> IndC (off the ctx critical path) ----
            nc.vector.reciprocal(recipZ5[0:4, :], z_ps[:])
            nc.vector.tensor_scalar_mul(IndC[:], ind5[:], recipZ5[:])

            # ---- ctx PSUM->SBUF, transpose to [128,4] slabs, c2 GEMM ----
            nc.scalar.copy(ctx_sb[:], ctx_ps[:])
            dummy_mm(3)
            for j in range(4):
                tr = ps_tr.tile([128, BLOC], bf16, tag="tr")
                nc.tensor.transpose(tr[:], ctx_sb[:, j * 128:(j + 1) * 128],
                                    id4)
                nc.vector.tensor_copy(ctxT[:, j * BLOC:(j + 1) * BLOC], tr[:])
            for j in range(4):
                nc.tensor.matmul(c2_ps[:], ctxT[:, j * BLOC:(j + 1) * BLOC],
                                 w3b[:, j * D:(j + 1) * D],
                                 start=(j == 0), stop=(j == 3))

            # rhs5 rows 0..3 = unnormalized c2 (split copy ACT/DVE);
            # row 4 = b3 (already there via the consts DMA).
            nc.scalar.copy(rhs5[0:4, 0:256], c2_ps[:, 0:256])
            nc.vector.tensor_copy(rhs5[0:4, 256:512], c2_ps[:, 256:512])
            dummy_mm(2)

            # ---- out += IndC^T @ [c2; b3] (normalization riding IndC),
            # two row halves; copies split by column across ACT/DVE ----
            H = BT // 2
            h0, h1 = slice(0, H), slice(H, BT)
            nc.tensor.matmul(out_ps[h0, :], IndC[:, h0], rhs5[:],
                             start=False, stop=False, skip_group_check=True)
            nc.tensor.matmul(out_ps[h1, :], IndC[:, h1], rhs5[:],
                             start=False, stop=True, skip_group_check=True)
            nc.scalar.copy(out_sb[h0, 0:256], out_ps[h0, 0:256])
            nc.vector.tensor_copy(out_sb[h0, 256:512], out_ps[h0, 256:512])
            nc.sync.dma_start(out_d[h0, :], out_sb[h0, :])
            nc.scalar.copy(out_sb[h1, 0:256], out_ps[h1, 0:256])
            nc.vector.tensor_copy(out_sb[h1, 256:512], out_ps[h1, 256:512])
            nc.gpsimd.dma_start(out_d[h1, :], out_sb[h1, :])

    nc.compile()
    return nc


def _get_graph():
    if "nc" not in _cache:
        _cache["nc"] = _build_graph()
    return _cache["nc"]


def _consts():
    if "consts" in _cache:
        return _cache["consts"]
    import ml_dtypes
    bf = ml_dtypes.bfloat16
    mask = np.zeros((CH, NCH, BLOC), np.float32)
    for c in range(NCH):
        for p in range(CH):
            mask[p, c, (c * CH + p) // L] = 1.0
    _cache["consts"] = {"_mask": mask.reshape(CH, MW)}
    return _cache["consts"]


def kernel(x, x_static, h0, W1, W2, W3, b2, b3, V, **_unused):
    import ml_dtypes
    from concourse.bass_utils import run_bass_kernel_spmd
    bf = ml_dtypes.bfloat16

    x = np.asarray(x, np.float32)
    x_static = np.asarray(x_static, np.float32)
    W1 = np.asarray(W1, np.float32)
    W3 = np.asarray(W3, np.float32)
    b3 = np.asarray(b3, np.float32)
    V = np.asarray(V, np.float32)

    # Host-side weight folding (weights are per-model constants).
    w1v = (W1 @ V).reshape(-1)                               # [S]
    w3t = (W3[:D].reshape(4, 128, D).transpose(1, 0, 2)
           .reshape(128, 4 * D))
    w3b = np.ascontiguousarray(
        (W3[D:] / w1v[:, None]).reshape(4, 128, D).transpose(1, 0, 2)
        .reshape(128, 4 * D).astype(bf))
    consts = _consts()
    cst = np.zeros((5, CW), np.float32)
    for b in range(BLOC):
        cst[b, b * T:(b + 1) * T] = 1.0                      # ind5 rows
    cst[4, 0:BT] = 1.0
    cst[0:4, 128:132] = np.eye(4)                            # id4
    cst[4, 132:132 + D] = b3                                 # b3 row
    cst = np.ascontiguousarray(cst.astype(bf))

    nc = _get_graph()
    in_maps = []
    for i in range(NCORES):
        sl = slice(i * BLOC, (i + 1) * BLOC)
        xsw = (x_static[sl].reshape(BL, S) * w1v[None, :])
        xs_p = xsw.reshape(NCH, CH, S).transpose(1, 0, 2)    # [CH, NCH, S]
        xsp = np.empty((CH, XSW), np.float32)
        for c in range(NCH):
            xsp[:, _CHOFF[c]:_CHOFF[c] + S] = xs_p[:, c]
        xsp[:, MASKOFF:MASKOFF + MW] = consts["_mask"]
        xsp = np.ascontiguousarray(xsp.astype(bf))
        xt_l = x[sl].reshape(BT, D).T                        # [512, 128]
        xt_p = (xt_l.reshape(4, 128, 128).transpose(1, 0, 2)
                .reshape(128, 512))
        w3tx = np.ascontiguousarray(
            np.concatenate([xt_p, w3t], axis=1).astype(bf))
        in_maps.append({
            "xsp": xsp, "w3tx": w3tx, "w3b": w3b, "cst": cst,
        })
    res = run_bass_kernel_spmd(nc, in_maps, core_ids=list(range(NCORES)))
    out = np.empty((B, T, D), np.float32)
    for i in range(NCORES):
        out[i * BLOC:(i + 1) * BLOC] = (
            res.results[i]["out"].astype(np.float32).reshape(BLOC, T, D))
    return out


# revision 8
# speedup vs baseline: 1.1325x; 1.0720x over previous
"""Bass/Trainium2 kernel for nn_Attentioncell (Bahdanau-style attention cell).

Mathematical simplification (rel-err ~6e-7 vs the jax reference): the
per-step scores are
    scores[b,l] = (total[b,l,:] + (h @ W2)[b,:]) @ V
               = (total @ V)[b,l] + (h @ W2 @ V)[b]
and softmax over l is invariant to the per-b shift, so the attention
weights are identical for every timestep and independent of h:
    attn = softmax_l(x_static @ (W1 @ V))        (b2, W2, h0 drop out)
    ctx[b,:] = sum_l attn[b,l] * x_static[b,l,:]
    out[b,t,:] = x[b,t,:] @ W3[:D] + ctx[b,:] @ W3[D:] + b3

Additional host-side weight folding: with w1v = W1@V,
    scores[l] = sum_s (x_static[l,s] * w1v[s])   = rowsum(xsw)
    ctx' = E^T @ xsw        (xsw = x_static * w1v, elementwise over s)
    ctx' @ (W3[D:] / w1v)  == ctx @ W3[D:]       (exact algebra)
so the device never multiplies by w1v: scores are plain row-sums of the
pre-scaled xsw, and W3bot is divided by w1v on the host.  Each product
term in the c2 GEMM has exactly the same magnitude as before, so the
bf16 error profile is unchanged (min |w1v| ~ 7e-5 -> max |W3bot'| ~2e3,
comfortably inside bf16 range).

Data-parallel over batch B=32 across 8 NeuronCores (4 per core).

Schedule notes (from perfetto trace analysis of the previous version):
  - PE has a DVFS p-state ramp (0.65 -> 1.2 -> 2.4 GHz after ~3us of
    continuous busy).  Dummy matmuls on a memset tile keep the PE busy
    through the ~2.6us DMA-latency head so real matmuls run at high
    clock; extra dummies are placed in known schedule bubbles.
  - 4 DMA queues (sync/scalar/gpsimd/vector), chunk order chosen by
    consumption deadline; per-DMA latency is ~2.2us fixed (descriptor
    gen + DGE start + completion-semaphore propagation) + transfer.
  - reduces: DVE tensor_reduce for chunks {0,2,4,5,6} (~.85us each),
    ACT Copy+accum for {1,3} (~1.2us each); exps in 4 waves on ACT;
    E-builds (exp * block-diagonal mask) on the otherwise-idle GpSimd.
  - Z = sum_l E via 7 tiny PE matmuls against a ones column riding the
    ctx accumulation; 1/Z is folded into the Ind5 indicator matrix
    (IndC = ind5 * recipZ) so the ctx PSUM->SBUF copy does not wait on
    the reciprocal.
  - out is produced in bf16 (halves the output DMA) and upconverted on
    the host; rel-err stays ~1e-3 vs the 2e-2 gate.
"""

import numpy as np

B, T, L, S, D = 32, 32, 196, 512, 512
NCORES = 8
BLOC = B // NCORES          # 4 batches per core
BT = BLOC * T               # 128 output rows per core
BL = BLOC * L               # 784 static rows per core
NCH = 7                     # xsw chunks
CH = BL // NCH              # 112 rows per chunk
MW = NCH * BLOC             # 28 mask columns

# xsp column layout: [c0 | c1 | mask | c2 | c3 | c4 | c5 | c6]
# (mask rides in the same DMA group as c1 so it lands early for the
# first E-build).
_CHOFF = [0, S, 2 * S + MW, 3 * S + MW, 4 * S + MW, 5 * S + MW, 6 * S + MW]
MASKOFF = 2 * S
XSW = 7 * S + MW            # 3612

# consts layout: [ind5 (128) | id4 (4) | rhs5 region (512)] on 5 partitions.
# rhs5 region row 4 = b3 (via DMA); rows 0..3 overwritten on-device by c2.
CW = 128 + 4 + 512

_cache = {}


def _build_graph():
    import concourse.bacc as bacc
    import concourse.tile as tile
    from concourse import mybir

    f32 = mybir.dt.float32
    bf16 = mybir.dt.bfloat16
    nc = bacc.Bacc("TRN2", target_bir_lowering=False, debug=False,
                   num_devices=NCORES)

    xs_d = nc.dram_tensor("xsp", [CH, XSW], bf16, kind="ExternalInput").ap()
    w3t_d = nc.dram_tensor("w3tx", [128, 512 + 4 * D], bf16,
                           kind="ExternalInput").ap()
    w3b_d = nc.dram_tensor("w3b", [128, 4 * D], bf16, kind="ExternalInput").ap()
    cst_d = nc.dram_tensor("cst", [5, CW], bf16, kind="ExternalInput").ap()
    out_d = nc.dram_tensor("out", [BT, D], bf16, kind="ExternalOutput").ap()

    with tile.TileContext(nc) as tc:
        with (
            tc.tile_pool(name="big", bufs=1) as big,
            tc.tile_pool(name="small", bufs=1) as small,
            tc.tile_pool(name="scratch", bufs=2) as scratch,
            tc.tile_pool(name="ps_acc", bufs=1, space="PSUM") as ps_acc,
            tc.tile_pool(name="ps_tr", bufs=2, space="PSUM") as ps_tr,
        ):
            xsp = big.tile([CH, XSW], bf16, tag="xsp")
            mask = xsp[:, MASKOFF:MASKOFF + MW]
            w3tx = big.tile([128, 512 + 4 * D], bf16, tag="w3tx")
            xt = w3tx[:, 0:512]
            w3t = w3tx[:, 512:]
            w3b = big.tile([128, 4 * D], bf16, tag="w3b")
            cst = small.tile([5, CW], bf16, tag="cst")
            ind5 = cst[:, 0:128]
            id4 = cst[0:4, 128:132]
            rhs5 = cst[:, 132:132 + D]
            dummy = big.tile([128, 512], bf16, tag="dummy")
            ones = small.tile([CH, 1], bf16, tag="ones")
            scores = small.tile([CH, NCH], f32, tag="scores")
            etile = small.tile([CH, NCH], bf16, tag="etile")
            E = small.tile([CH, MW], bf16, tag="E")
            recipZ5 = small.tile([5, 1], f32, tag="recipZ5")
            IndC = small.tile([5, BT], bf16, tag="IndC")
            ctx_sb = small.tile([BLOC, S], bf16, tag="ctx_sb")
            ctxT = small.tile([128, 4 * BLOC], bf16, tag="ctxT")
            out_sb = big.tile([BT, D], bf16, tag="out_sb")

            # ---- memsets first (dummy feeds the PE warmup stream; on DVE
            # so the GpSimd DMA queue starts immediately) ----
            nc.vector.memset(dummy[:], 0.0)
            nc.vector.memset(recipZ5[:], 1.0)   # rows 0..3 overwritten by recip

            # ---- DMA loads. The 3 queues (SP/ACT/GpSimd) share ~300GB/s
            # of HBM and each queue processes its list serially, so the
            # global arrival order is round-based across queues: round 1-2
            # carry all score chunks, later rounds carry the GEMM weights
            # (w3b before w3t: the c2 chain consumes w3b ~1.5us before the
            # final matmuls need w3t). ----
            def chunk(c):
                return xsp[:, _CHOFF[c]:_CHOFF[c] + S]

            nc.sync.dma_start(xsp[:, 0:S], xs_d[:, 0:S])                 # c0
            nc.scalar.dma_start(xsp[:, S:S + S + MW],
                                xs_d[:, S:S + S + MW])                   # c1+mask
            nc.gpsimd.dma_start(xsp[:, _CHOFF[2]:_CHOFF[2] + S],
                                xs_d[:, _CHOFF[2]:_CHOFF[2] + S])        # c2
            nc.sync.dma_start(xsp[:, _CHOFF[3]:_CHOFF[3] + 2 * S],
                              xs_d[:, _CHOFF[3]:_CHOFF[3] + 2 * S])      # c3+c4
            nc.scalar.dma_start(xsp[:, _CHOFF[5]:_CHOFF[5] + 2 * S],
                                xs_d[:, _CHOFF[5]:_CHOFF[5] + 2 * S])    # c5+c6
            nc.gpsimd.dma_start(cst[:], cst_d[:])                        # consts
            nc.gpsimd.dma_start(w3tx[:, 0:1024], w3t_d[:, 0:1024])       # xt+w3t0
            nc.sync.dma_start(w3b[:, 0:2 * D], w3b_d[:, 0:2 * D])        # w3b01
            nc.scalar.dma_start(w3b[:, 2 * D:], w3b_d[:, 2 * D:])        # w3b23
            nc.gpsimd.dma_start(w3tx[:, 1024:1024 + 2 * D],
                                w3t_d[:, 1024:1024 + 2 * D])             # w3t12
            nc.sync.dma_start(w3tx[:, 1024 + 2 * D:],
                              w3t_d[:, 1024 + 2 * D:])                   # w3t3
            nc.gpsimd.memset(ones[:], 1.0)

            out_ps = ps_acc.tile([BT, D], f32, tag="out_ps")
            ctx_ps = ps_acc.tile([BLOC, S], f32, tag="ctx_ps")
            z_ps = ps_acc.tile([BLOC, 1], f32, tag="z_ps")
            c2_ps = ps_acc.tile([BLOC, D], f32, tag="c2_ps")
            dm_ps = ps_acc.tile([128, 512], f32, tag="dm_ps")

            # ---- PE warmup: keep the array busy through the DMA head so
            # the p-state ramps to full clock before real work arrives ----
            def dummy_mm(n):
                for _ in range(n):
                    nc.tensor.matmul(dm_ps[:], dummy[:, 0:128], dummy[:],
                                     start=True, stop=True,
                                     skip_group_check=True)

            dummy_mm(10)

            # ---- score reduces: DVE {0,2,4,5} + half of 6 (GpSimd
            # pre-folds c6 into a [112,256] pair-sum), ACT {1,3} ----
            c6sum = scratch.tile([CH, 256], bf16, tag="c6sum")

            def dve_reduce(c):
                nc.vector.tensor_reduce(scores[:, c:c + 1], chunk(c),
                                        axis=mybir.AxisListType.X,
                                        op=mybir.AluOpType.add)

            def act_reduce(c):
                dump = scratch.tile([CH, S], bf16, tag="dump")
                nc.scalar.activation(dump[:], chunk(c),
                                     mybir.ActivationFunctionType.Copy,
                                     accum_out=scores[:, c:c + 1])

            def exp_wave(a, b):
                nc.scalar.activation(etile[:, a:b], scores[:, a:b],
                                     mybir.ActivationFunctionType.Exp)

            def ebuild(a, b):
                n = b - a
                nc.gpsimd.tensor_mul(
                    E[:, a * BLOC:b * BLOC].rearrange(
                        "p (c b) -> p c b", b=BLOC),
                    etile[:, a:b].to_broadcast((CH, n, BLOC)),
                    mask[:, a * BLOC:b * BLOC].rearrange(
                        "p (c b) -> p c b", b=BLOC),
                )

            def ctx_mm(c):
                nc.tensor.matmul(ctx_ps[:], E[:, c * BLOC:(c + 1) * BLOC],
                                 chunk(c), start=(c == 0), stop=(c == NCH - 1))
                nc.tensor.matmul(z_ps[:], E[:, c * BLOC:(c + 1) * BLOC],
                                 ones[:], start=(c == 0), stop=(c == NCH - 1))

            def out_top(j):
                nc.tensor.matmul(out_ps[:], xt[:, j * 128:(j + 1) * 128],
                                 w3t[:, j * D:(j + 1) * D],
                                 start=(j == 0), stop=False,
                                 skip_group_check=True)

            # DVE chain
            dve_reduce(0)
            dve_reduce(2)
            dve_reduce(4)
            dve_reduce(5)
            # GpSimd pre-fold of c6 (pair-sum halves), then DVE half-reduce
            nc.gpsimd.tensor_add(c6sum[:], chunk(6)[:, 0:256],
                                 chunk(6)[:, 256:512])
            nc.vector.tensor_reduce(scores[:, 6:7], c6sum[:],
                                    axis=mybir.AxisListType.X,
                                    op=mybir.AluOpType.add)
            # ACT chain (exps interleaved so none is blocked by a reduce)
            act_reduce(1)
            exp_wave(0, 2)
            act_reduce(3)
            exp_wave(2, 4)
            exp_wave(4, 6)
            exp_wave(6, 7)
            # GpSimd E-builds
            ebuild(0, 2)
            ebuild(2, 4)
            ebuild(4, 6)
            ebuild(6, 7)
            # PE stream (emission order ~= expected readiness)
            ctx_mm(0)
            ctx_mm(1)
            dummy_mm(2)
            ctx_mm(2)
            ctx_mm(3)
            out_top(0)
            ctx_mm(4)
            ctx_mm(5)
            ctx_mm(6)
            dummy_mm(1)

            # ---- 1/Z -> IndC (off the ctx critical path; IndC on GpSimd) ----
            nc.vector.reciprocal(recipZ5[0:4, :], z_ps[:])
            nc.gpsimd.tensor_scalar_mul(IndC[:], ind5[:], recipZ5[:])

            # ---- ctx PSUM->SBUF (col-split ACT/DVE), transpose, c2 GEMM ----
            nc.scalar.copy(ctx_sb[:, 0:256], ctx_ps[:, 0:256])
            nc.vector.tensor_copy(ctx_sb[:, 256:512], ctx_ps[:, 256:512])
            for j in range(4):
                tr = ps_tr.tile([128, BLOC], bf16, tag="tr")
                nc.tensor.transpose(tr[:], ctx_sb[:, j * 128:(j + 1) * 128],
                                    id4)
                nc.vector.tensor_copy(ctxT[:, j * BLOC:(j + 1) * BLOC], tr[:])
            out_top(1)
            for j in range(4):
                nc.tensor.matmul(c2_ps[:], ctxT[:, j * BLOC:(j + 1) * BLOC],
                                 w3b[:, j * D:(j + 1) * D],
                                 start=(j == 0), stop=(j == 3))
                if j == 1:
                    out_top(2)
            out_top(3)

            # rhs5 rows 0..3 = unnormalized c2 (split copy ACT/DVE);
            # row 4 = b3 (already there via the consts DMA).
            nc.scalar.copy(rhs5[0:4, 0:256], c2_ps[:, 0:256])
            nc.vector.tensor_copy(rhs5[0:4, 256:512], c2_ps[:, 256:512])

            # ---- out += IndC^T @ [c2; b3] (normalization riding IndC),
            # two row halves; copies split by column across ACT/DVE ----
            H = BT // 2
            h0, h1 = slice(0, H), slice(H, BT)
            nc.tensor.matmul(out_ps[h0, :], IndC[:, h0], rhs5[:],
                             start=False, stop=False, skip_group_check=True)
            nc.tensor.matmul(out_ps[h1, :], IndC[:, h1], rhs5[:],
                             start=False, stop=True, skip_group_check=True)
            nc.scalar.copy(out_sb[h0, 0:256], out_ps[h0, 0:256])
            nc.vector.tensor_copy(out_sb[h0, 256:512], out_ps[h0, 256:512])
            nc.sync.dma_start(out_d[h0, :], out_sb[h0, :])
            nc.scalar.copy(out_sb[h1, 0:256], out_ps[h1, 0:256])
            nc.vector.tensor_copy(out_sb[h1, 256:512], out_ps[h1, 256:512])
            nc.gpsimd.dma_start(out_d[h1, :], out_sb[h1, :])

    nc.compile()
    return nc


def _get_graph():
    if "nc" not in _cache:
        _cache["nc"] = _build_graph()
    return _cache["nc"]


def _consts():
    if "consts" in _cache:
        return _cache["consts"]
    import ml_dtypes
    bf = ml_dtypes.bfloat16
    mask = np.zeros((CH, NCH, BLOC), np.float32)
    for c in range(NCH):
        for p in range(CH):
            mask[p, c, (c * CH + p) // L] = 1.0
    _cache["consts"] = {"_mask": mask.reshape(CH, MW)}
    return _cache["consts"]


def kernel(x, x_static, h0, W1, W2, W3, b2, b3, V, **_unused):
    import ml_dtypes
    from concourse.bass_utils import run_bass_kernel_spmd
    bf = ml_dtypes.bfloat16

    x = np.asarray(x, np.float32)
    x_static = np.asarray(x_static, np.float32)
    W1 = np.asarray(W1, np.float32)
    W3 = np.asarray(W3, np.float32)
    b3 = np.asarray(b3, np.float32)
    V = np.asarray(V, np.float32)

    # Host-side weight folding (weights are per-model constants).
    w1v = (W1 @ V).reshape(-1)                               # [S]
    w3t = (W3[:D].reshape(4, 128, D).transpose(1, 0, 2)
           .reshape(128, 4 * D))
    w3b = np.ascontiguousarray(
        (W3[D:] / w1v[:, None]).reshape(4, 128, D).transpose(1, 0, 2)
        .reshape(128, 4 * D).astype(bf))
    consts = _consts()
    cst = np.zeros((5, CW), np.float32)
    for b in range(BLOC):
        cst[b, b * T:(b + 1) * T] = 1.0                      # ind5 rows
    cst[4, 0:BT] = 1.0
    cst[0:4, 128:132] = np.eye(4)                            # id4
    cst[4, 132:132 + D] = b3                                 # b3 row
    cst = np.ascontiguousarray(cst.astype(bf))

    nc = _get_graph()
    in_maps = []
    for i in range(NCORES):
        sl = slice(i * BLOC, (i + 1) * BLOC)
        xsw = (x_static[sl].reshape(BL, S) * w1v[None, :])
        xs_p = xsw.reshape(NCH, CH, S).transpose(1, 0, 2)    # [CH, NCH, S]
        xsp = np.empty((CH, XSW), np.float32)
        for c in range(NCH):
            xsp[:, _CHOFF[c]:_CHOFF[c] + S] = xs_p[:, c]
        xsp[:, MASKOFF:MASKOFF + MW] = consts["_mask"]
        xsp = np.ascontiguousarray(xsp.astype(bf))
        xt_l = x[sl].reshape(BT, D).T                        # [512, 128]
        xt_p = (xt_l.reshape(4, 128, 128).transpose(1, 0, 2)
                .reshape(128, 512))
        w3tx = np.ascontiguousarray(
            np.concatenate([xt_p, w3t], axis=1).astype(bf))
        in_maps.append({
            "xsp": xsp, "w3tx": w3tx, "w3b": w3b, "cst": cst,
        })
    res = run_bass_kernel_spmd(nc, in_maps, core_ids=list(range(NCORES)))
    out = np.empty((B, T, D), np.float32)
    for i in range(NCORES):
        out[i * BLOC:(i + 1) * BLOC] = (
            res.results[i]["out"].astype(np.float32).reshape(BLOC, T, D))
    return out


# revision 13
# speedup vs baseline: 1.1388x; 1.0056x over previous
"""Bass/Trainium2 kernel for nn_Attentioncell (Bahdanau-style attention cell).

Mathematical simplification (rel-err ~6e-7 vs the jax reference): the
per-step scores are
    scores[b,l] = (total[b,l,:] + (h @ W2)[b,:]) @ V
               = (total @ V)[b,l] + (h @ W2 @ V)[b]
and softmax over l is invariant to the per-b shift, so the attention
weights are identical for every timestep and independent of h:
    attn = softmax_l(x_static @ (W1 @ V))        (b2, W2, h0 drop out)
    ctx[b,:] = sum_l attn[b,l] * x_static[b,l,:]
    out[b,t,:] = x[b,t,:] @ W3[:D] + ctx[b,:] @ W3[D:] + b3

Additional host-side weight folding: with w1v = W1@V,
    scores[l] = sum_s (x_static[l,s] * w1v[s])   = rowsum(xsw)
    ctx' = E^T @ xsw        (xsw = x_static * w1v, elementwise over s)
    ctx' @ (W3[D:] / w1v)  == ctx @ W3[D:]       (exact algebra)
so the device never multiplies by w1v: scores are plain row-sums of the
pre-scaled xsw, and W3bot is divided by w1v on the host.  Each product
term in the c2 GEMM has exactly the same magnitude as before, so the
bf16 error profile is unchanged (min |w1v| ~ 7e-5 -> max |W3bot'| ~2e3,
comfortably inside bf16 range).

Data-parallel over batch B=32 across 8 NeuronCores (4 per core).

Schedule notes (from perfetto trace analysis of the previous version):
  - PE has a DVFS p-state ramp (0.65 -> 1.2 -> 2.4 GHz after ~3us of
    continuous busy).  Dummy matmuls on a memset tile keep the PE busy
    through the ~2.6us DMA-latency head so real matmuls run at high
    clock; extra dummies are placed in known schedule bubbles.
  - 4 DMA queues (sync/scalar/gpsimd/vector), chunk order chosen by
    consumption deadline; per-DMA latency is ~2.2us fixed (descriptor
    gen + DGE start + completion-semaphore propagation) + transfer.
  - reduces: DVE tensor_reduce for chunks {0,2,4,5,6} (~.85us each),
    ACT Copy+accum for {1,3} (~1.2us each); exps in 4 waves on ACT;
    E-builds (exp * block-diagonal mask) on the otherwise-idle GpSimd.
  - Z = sum_l E via 7 tiny PE matmuls against a ones column riding the
    ctx accumulation; 1/Z is folded into the Ind5 indicator matrix
    (IndC = ind5 * recipZ) so the ctx PSUM->SBUF copy does not wait on
    the reciprocal.
  - out is produced in bf16 (halves the output DMA) and upconverted on
    the host; rel-err stays ~1e-3 vs the 2e-2 gate.
"""

import numpy as np

B, T, L, S, D = 32, 32, 196, 512, 512
NCORES = 8
BLOC = B // NCORES          # 4 batches per core
BT = BLOC * T               # 128 output rows per core
BL = BLOC * L               # 784 static rows per core
NCH = 7                     # xsw chunks
CH = BL // NCH              # 112 rows per chunk
MW = NCH * BLOC             # 28 mask columns

# xsp column layout: [c0 | c1 | mask | c2 | c3 | c4 | c5 | c6]
# (mask rides in the same DMA group as c1/c2 so it lands early for the
# first E-build).
_CHOFF = [0, S, 2 * S + MW, 3 * S + MW, 4 * S + MW, 5 * S + MW, 6 * S + MW]
MASKOFF = 2 * S
XSW = 7 * S + MW            # 3612

# score-column s <-> chunk SCORD[s]: columns ordered by expected DMA
# arrival (c5/c6 ride the GpSimd queue first DMA, c3/c4 land last) so
# reduces/exp waves run in arrival order with contiguous column slices.
SCORD = [0, 1, 2, 5, 6, 3, 4]

# consts layout: [ind5 (128) | id4 (4) | rhs5 region (512)] on 5 partitions.
# rhs5 region row 4 = b3 (via DMA); rows 0..3 overwritten on-device by c2.
CW = 128 + 4 + 512

_cache = {}


def _build_graph():
    import concourse.bacc as bacc
    import concourse.tile as tile
    from concourse import mybir

    f32 = mybir.dt.float32
    bf16 = mybir.dt.bfloat16
    nc = bacc.Bacc("TRN2", target_bir_lowering=False, debug=False,
                   num_devices=NCORES)

    xs_d = nc.dram_tensor("xsp", [CH, XSW], bf16, kind="ExternalInput").ap()
    w3t_d = nc.dram_tensor("w3tx", [128, 512 + 4 * D], bf16,
                           kind="ExternalInput").ap()
    w3b_d = nc.dram_tensor("w3b", [128, 4 * D], bf16, kind="ExternalInput").ap()
    cst_d = nc.dram_tensor("cst", [5, CW], bf16, kind="ExternalInput").ap()
    out_d = nc.dram_tensor("out", [BT, D], bf16, kind="ExternalOutput").ap()

    with tile.TileContext(nc) as tc:
        with (
            tc.tile_pool(name="big", bufs=1) as big,
            tc.tile_pool(name="small", bufs=1) as small,
            tc.tile_pool(name="scratch", bufs=2) as scratch,
            tc.tile_pool(name="ps_acc", bufs=1, space="PSUM") as ps_acc,
            tc.tile_pool(name="ps_tr", bufs=2, space="PSUM") as ps_tr,
        ):
            xsp = big.tile([CH, XSW], bf16, tag="xsp")
            mask = xsp[:, MASKOFF:MASKOFF + MW]
            w3tx = big.tile([128, 512 + 4 * D], bf16, tag="w3tx")
            xt = w3tx[:, 0:512]
            w3t = w3tx[:, 512:]
            w3b = big.tile([128, 4 * D], bf16, tag="w3b")
            cst = small.tile([5, CW], bf16, tag="cst")
            ind5 = cst[:, 0:128]
            id4 = cst[0:4, 128:132]
            rhs5 = cst[:, 132:132 + D]
            dummy = big.tile([128, 512], bf16, tag="dummy")
            ones = small.tile([CH, 1], bf16, tag="ones")
            scores = small.tile([CH, NCH], f32, tag="scores")
            etile = small.tile([CH, NCH], bf16, tag="etile")
            E = small.tile([CH, MW], bf16, tag="E")
            recipZ5 = small.tile([5, 1], f32, tag="recipZ5")
            IndC = small.tile([5, BT], bf16, tag="IndC")
            ctx_sb = small.tile([BLOC, S], bf16, tag="ctx_sb")
            ctxT = small.tile([128, 4 * BLOC], bf16, tag="ctxT")
            out_sb = big.tile([BT, D], bf16, tag="out_sb")

            # ---- memsets first (dummy feeds the PE warmup stream; on DVE
            # so the GpSimd DMA queue starts immediately) ----
            nc.vector.memset(dummy[:], 0.0)
            nc.vector.memset(recipZ5[:], 1.0)   # rows 0..3 overwritten by recip

            # ---- DMA loads. The 3 queues (SP/ACT/GpSimd) share ~300GB/s
            # of HBM and each queue processes its list serially, so the
            # global arrival order is round-based across queues: round 1-2
            # carry all score chunks, later rounds carry the GEMM weights
            # (w3b before w3t: the c2 chain consumes w3b ~1.5us before the
            # final matmuls need w3t). ----
            def chunk(c):
                return xsp[:, _CHOFF[c]:_CHOFF[c] + S]

            nc.sync.dma_start(xsp[:, 0:S], xs_d[:, 0:S])                 # c0
            nc.scalar.dma_start(xsp[:, S:_CHOFF[2] + S],
                                xs_d[:, S:_CHOFF[2] + S])                # c1+mask+c2
            nc.gpsimd.dma_start(xsp[:, _CHOFF[5]:_CHOFF[5] + 2 * S],
                                xs_d[:, _CHOFF[5]:_CHOFF[5] + 2 * S])    # c5+c6
            nc.sync.dma_start(xsp[:, _CHOFF[3]:_CHOFF[3] + 2 * S],
                              xs_d[:, _CHOFF[3]:_CHOFF[3] + 2 * S])      # c3+c4
            nc.gpsimd.dma_start(cst[:], cst_d[:])                        # consts
            nc.sync.dma_start(w3b[:, 0:2 * D], w3b_d[:, 0:2 * D])        # w3b01
            nc.scalar.dma_start(w3b[:, 2 * D:], w3b_d[:, 2 * D:])        # w3b23
            nc.gpsimd.dma_start(w3tx[:, 0:1024], w3t_d[:, 0:1024])       # xt+w3t0
            nc.scalar.dma_start(w3tx[:, 1024:1024 + 2 * D],
                                w3t_d[:, 1024:1024 + 2 * D])             # w3t12
            nc.sync.dma_start(w3tx[:, 1024 + 2 * D:],
                              w3t_d[:, 1024 + 2 * D:])                   # w3t3
            nc.gpsimd.memset(ones[:], 1.0)

            out_ps = ps_acc.tile([BT, D], f32, tag="out_ps")
            ctx_ps = ps_acc.tile([BLOC, S], f32, tag="ctx_ps")
            z_ps = ps_acc.tile([BLOC, 1], f32, tag="z_ps")
            c2_ps = ps_acc.tile([BLOC, D], f32, tag="c2_ps")
            dm_ps = ps_acc.tile([128, 512], f32, tag="dm_ps")

            # ---- PE warmup: keep the array busy through the DMA head so
            # the p-state ramps to full clock before real work arrives ----
            def dummy_mm(n):
                for _ in range(n):
                    nc.tensor.matmul(dm_ps[:], dummy[:, 0:128], dummy[:],
                                     start=True, stop=True,
                                     skip_group_check=True)

            dummy_mm(10)

            # ---- score reduces, in score-column (= arrival) order.
            # scores[:, s] = rowsum(chunk SCORD[s]); DVE does most, ACT
            # two, GpSimd pre-folds c6 into a [112,256] pair-sum. ----
            c6sum = scratch.tile([CH, 256], bf16, tag="c6sum")

            def dve_reduce(s):
                nc.vector.tensor_reduce(scores[:, s:s + 1], chunk(SCORD[s]),
                                        axis=mybir.AxisListType.X,
                                        op=mybir.AluOpType.add)

            def act_reduce(s):
                dump = scratch.tile([CH, S], bf16, tag="dump")
                nc.scalar.activation(dump[:], chunk(SCORD[s]),
                                     mybir.ActivationFunctionType.Copy,
                                     accum_out=scores[:, s:s + 1])

            def exp_wave(a, b):
                nc.scalar.activation(etile[:, a:b], scores[:, a:b],
                                     mybir.ActivationFunctionType.Exp)

            def ebuild(a, b):
                n = b - a
                nc.gpsimd.tensor_mul(
                    E[:, a * BLOC:b * BLOC].rearrange(
                        "p (c b) -> p c b", b=BLOC),
                    etile[:, a:b].to_broadcast((CH, n, BLOC)),
                    mask[:, a * BLOC:b * BLOC].rearrange(
                        "p (c b) -> p c b", b=BLOC),
                )

            def ctx_mm(s):
                nc.tensor.matmul(ctx_ps[:], E[:, s * BLOC:(s + 1) * BLOC],
                                 chunk(SCORD[s]), start=(s == 0),
                                 stop=(s == NCH - 1))
                nc.tensor.matmul(z_ps[:], E[:, s * BLOC:(s + 1) * BLOC],
                                 ones[:], start=(s == 0), stop=(s == NCH - 1))

            def out_top(j):
                nc.tensor.matmul(out_ps[:], xt[:, j * 128:(j + 1) * 128],
                                 w3t[:, j * D:(j + 1) * D],
                                 start=(j == 0), stop=False,
                                 skip_group_check=True)

            # DVE chain: score cols 0 (c0), 2 (c2), 3 (c5), 4 (c6 half), 6 (c4)
            dve_reduce(0)
            dve_reduce(2)
            dve_reduce(3)
            # GpSimd pre-fold of c6 (pair-sum halves), then DVE half-reduce
            nc.gpsimd.tensor_add(c6sum[:], chunk(6)[:, 0:256],
                                 chunk(6)[:, 256:512])
            nc.vector.tensor_reduce(scores[:, 4:5], c6sum[:],
                                    axis=mybir.AxisListType.X,
                                    op=mybir.AluOpType.add)
            dve_reduce(6)
            # ACT chain: score cols 1 (c1), 5 (c3); exps interleaved
            act_reduce(1)
            exp_wave(0, 2)
            exp_wave(2, 4)
            act_reduce(5)
            exp_wave(4, 6)
            exp_wave(6, 7)
            # GpSimd E-builds
            ebuild(0, 2)
            ebuild(2, 4)
            ebuild(4, 6)
            ebuild(6, 7)
            # PE stream (emission order ~= expected readiness)
            ctx_mm(0)
            ctx_mm(1)
            ctx_mm(2)
            ctx_mm(3)
            dummy_mm(1)
            ctx_mm(4)
            ctx_mm(5)
            ctx_mm(6)
            out_top(0)
            dummy_mm(1)

            # ---- 1/Z -> IndC (off the ctx critical path) ----
            nc.vector.reciprocal(recipZ5[0:4, :], z_ps[:])
            nc.vector.tensor_scalar_mul(IndC[:], ind5[:], recipZ5[:])

            # ---- ctx PSUM->SBUF (col-split ACT/DVE), transpose, c2 GEMM ----
            nc.scalar.copy(ctx_sb[:, 0:256], ctx_ps[:, 0:256])
            nc.vector.tensor_copy(ctx_sb[:, 256:512], ctx_ps[:, 256:512])
            for j in range(4):
                tr = ps_tr.tile([128, BLOC], bf16, tag="tr")
                nc.tensor.transpose(tr[:], ctx_sb[:, j * 128:(j + 1) * 128],
                                    id4)
                nc.vector.tensor_copy(ctxT[:, j * BLOC:(j + 1) * BLOC], tr[:])
            out_top(1)
            for j in range(4):
                nc.tensor.matmul(c2_ps[:], ctxT[:, j * BLOC:(j + 1) * BLOC],
                                 w3b[:, j * D:(j + 1) * D],
                                 start=(j == 0), stop=(j == 3))
                if j == 1:
                    out_top(2)
            out_top(3)

            # rhs5 rows 0..3 = unnormalized c2 (split copy ACT/DVE);
            # row 4 = b3 (already there via the consts DMA).
            nc.scalar.copy(rhs5[0:4, 0:256], c2_ps[:, 0:256])
            nc.vector.tensor_copy(rhs5[0:4, 256:512], c2_ps[:, 256:512])

            # ---- out += IndC^T @ [c2; b3] (normalization riding IndC),
            # two row halves; copies split by column across ACT/DVE ----
            H = BT // 2
            h0, h1 = slice(0, H), slice(H, BT)
            nc.tensor.matmul(out_ps[h0, :], IndC[:, h0], rhs5[:],
                             start=False, stop=False, skip_group_check=True)
            nc.tensor.matmul(out_ps[h1, :], IndC[:, h1], rhs5[:],
                             start=False, stop=True, skip_group_check=True)
            nc.scalar.copy(out_sb[h0, 0:256], out_ps[h0, 0:256])
            nc.vector.tensor_copy(out_sb[h0, 256:512], out_ps[h0, 256:512])
            nc.sync.dma_start(out_d[h0, :], out_sb[h0, :])
            nc.scalar.copy(out_sb[h1, 0:256], out_ps[h1, 0:256])
            nc.vector.tensor_copy(out_sb[h1, 256:512], out_ps[h1, 256:512])
            nc.scalar.dma_start(out_d[h1, :], out_sb[h1, :])

    nc.compile()
    return nc


def _get_graph():
    if "nc" not in _cache:
        _cache["nc"] = _build_graph()
    return _cache["nc"]


def _consts():
    if "consts" in _cache:
        return _cache["consts"]
    import ml_dtypes
    bf = ml_dtypes.bfloat16
    # mask column group s corresponds to chunk SCORD[s]
    mask = np.zeros((CH, NCH, BLOC), np.float32)
    for s in range(NCH):
        c = SCORD[s]
        for p in range(CH):
            mask[p, s, (c * CH + p) // L] = 1.0
    _cache["consts"] = {"_mask": mask.reshape(CH, MW)}
    return _cache["consts"]


def kernel(x, x_static, h0, W1, W2, W3, b2, b3, V, **_unused):
    import ml_dtypes
    from concourse.bass_utils import run_bass_kernel_spmd
    bf = ml_dtypes.bfloat16

    x = np.asarray(x, np.float32)
    x_static = np.asarray(x_static, np.float32)
    W1 = np.asarray(W1, np.float32)
    W3 = np.asarray(W3, np.float32)
    b3 = np.asarray(b3, np.float32)
    V = np.asarray(V, np.float32)

    # Host-side weight folding (weights are per-model constants).
    w1v = (W1 @ V).reshape(-1)                               # [S]
    w3t = (W3[:D].reshape(4, 128, D).transpose(1, 0, 2)
           .reshape(128, 4 * D))
    w3b = np.ascontiguousarray(
        (W3[D:] / w1v[:, None]).reshape(4, 128, D).transpose(1, 0, 2)
        .reshape(128, 4 * D).astype(bf))
    consts = _consts()
    cst = np.zeros((5, CW), np.float32)
    for b in range(BLOC):
        cst[b, b * T:(b + 1) * T] = 1.0                      # ind5 rows
    cst[4, 0:BT] = 1.0
    cst[0:4, 128:132] = np.eye(4)                            # id4
    cst[4, 132:132 + D] = b3                                 # b3 row
    cst = np.ascontiguousarray(cst.astype(bf))

    nc = _get_graph()
    in_maps = []
    for i in range(NCORES):
        sl = slice(i * BLOC, (i + 1) * BLOC)
        xsw = (x_static[sl].reshape(BL, S) * w1v[None, :])
        xs_p = xsw.reshape(NCH, CH, S).transpose(1, 0, 2)    # [CH, NCH, S]
        xsp = np.empty((CH, XSW), np.float32)
        for c in range(NCH):
            xsp[:, _CHOFF[c]:_CHOFF[c] + S] = xs_p[:, c]
        xsp[:, MASKOFF:MASKOFF + MW] = consts["_mask"]
        xsp = np.ascontiguousarray(xsp.astype(bf))
        xt_l = x[sl].reshape(BT, D).T                        # [512, 128]
        xt_p = (xt_l.reshape(4, 128, 128).transpose(1, 0, 2)
                .reshape(128, 512))
        w3tx = np.ascontiguousarray(
            np.concatenate([xt_p, w3t], axis=1).astype(bf))
        in_maps.append({
            "xsp": xsp, "w3tx": w3tx, "w3b": w3b, "cst": cst,
        })
    res = run_bass_kernel_spmd(nc, in_maps, core_ids=list(range(NCORES)))
    out = np.empty((B, T, D), np.float32)
    for i in range(NCORES):
        out[i * BLOC:(i + 1) * BLOC] = (
            res.results[i]["out"].astype(np.float32).reshape(BLOC, T, D))
    return out
